# revision 10
# baseline (speedup 1.0000x reference)
"""Trainium2 Bass kernel for a dense transformer encoder layer.

Shapes: B=4, S=2048, D=512, H=8 heads (HD=64), FFN F=2048.

Sharding (8 NeuronCores, no collectives): core c handles batch b = c//2 and
query-half half = c%2 (1024 query tokens); K/V are computed for the full
2048-token sequence on both cores of a pair.

Design notes:
- The attention phase is ACT-bound: the 128 softmax exp ops ([128,1024] each)
  set a ~135us floor. Everything else is scheduled under that roof.
- All attention-path matmuls (QKV proj, scores, AV, Wo) and FFN1 run as fp8e4
  DoubleRow matmuls (2 contraction tiles per instruction, 0.5 cycles/row).
  Scores use a duplicated-plane trick (both planes hold K/sqrt2, Q/sqrt2) to
  fill the 2x64 contraction; projections pair genuine 128-deep c-chunks.
- AV keeps exp stationary-free: lhsT = V8 [key,2,65] over kc pairs, rhs = the
  exp pair [128,2,1024]; the softmax denominator rides in V column 64.
- The residual spine (res1/res2) is fp32 to buy error headroom for fp8;
  LayerNorm stats run off bf16 copies via ones-matmuls.
- FFN2 stays bf16 (fp8 there costs too much accuracy).
- Host transposes x / un-transposes the output.
"""

import functools
import numpy as np
from contextlib import ExitStack

import ml_dtypes

import concourse.bass as bass
import concourse.tile as tile
import concourse.mybir as mybir
from concourse import bacc
from concourse.bass import ts
from concourse.vector_clock import ScopedClock

B, S, D, H, F = 4, 2048, 512, 8, 2048
HD = D // H           # 64
P = 128
DC = D // P           # 4   d chunks
FC = F // P           # 16  ffn chunks
SC = S // P           # 16  key chunks
KCP = SC // 2         # 8   key-chunk pairs
TOK = S // 2          # 1024 query tokens per core
EPS = 1e-5
VW = HD + 2           # 66: V cols per head incl. ones + pad (DR needs even)

f32 = mybir.dt.float32
bf16 = mybir.dt.bfloat16
f8 = mybir.dt.float8e4
AF = mybir.ActivationFunctionType
ALU = mybir.AluOpType
DR = mybir.MatmulPerfMode.DoubleRow

F8NP = ml_dtypes.float8_e4m3


class _TC(tile.TileContext):
    """TileContext whose tail drain splits sem waits one-per-drain: the
    walrus build in this container rejects >1 sync wait on an SP TPB_CTRL."""

    def _drain_and_barrier(self, tick_clock, wait_clock):
        nc = self.nc
        drain_inst = nc.sync.drain()
        wait_clock.add_sem_waits(
            drain_inst.ins, ScopedClock({None: tick_clock.global_clock})
        )
        si = drain_inst.ins.sync_info
        waits = list(si.on_wait) if si and si.on_wait else []
        MAXW = 1
        if len(waits) > MAXW:
            si.on_wait = waits[:MAXW]
            for i in range(MAXW, len(waits), MAXW):
                extra = nc.sync.drain()
                extra.ins.sync_info = mybir.SyncInfo(
                    on_wait=waits[i : i + MAXW], on_update=[]
                )
        nc.all_engine_barrier()
        popped = nc._tile_sem_poison_stack.pop()
        assert popped is self._sem_poison
        nc.clear_and_free_semaphores(list(self.sems.allocated().values()))
        nc.all_engine_barrier()


def _bcast_ap(row_ap, nparts):
    """AP reading the single-partition row `row_ap` broadcast to nparts."""
    return bass.AP(
        tensor=row_ap.tensor,
        offset=row_ap.offset,
        ap=[[0, nparts]] + [list(d) for d in row_ap.ap[1:]],
    )


def _ln_alloc(nc, stat_pool, tagp):
    return {
        "mu": stat_pool.tile([1, TOK], f32, name=f"{tagp}_mu", tag=f"{tagp}_mu"),
        "tB": stat_pool.tile([1, TOK], f32, name=f"{tagp}_tB", tag=f"{tagp}_tB"),
        "var": stat_pool.tile([1, TOK], f32, name=f"{tagp}_var", tag=f"{tagp}_var"),
        "a": stat_pool.tile([1, TOK], f32, name=f"{tagp}_a", tag=f"{tagp}_a"),
        "b": stat_pool.tile([1, TOK], f32, name=f"{tagp}_b", tag=f"{tagp}_b"),
        "abf": stat_pool.tile([1, TOK], bf16, name=f"{tagp}_abf",
                              tag=f"{tagp}_abf"),
        "bbf": stat_pool.tile([1, TOK], bf16, name=f"{tagp}_bbf",
                              tag=f"{tagp}_bbf"),
        "bcA": stat_pool.tile([P, TOK], bf16, name=f"{tagp}_bcA",
                              tag=f"{tagp}_bcA"),
        "bcB": stat_pool.tile([P, TOK], bf16, name=f"{tagp}_bcB",
                              tag=f"{tagp}_bcB"),
    }


def _ln_chain(nc, t, sum_ps, sq_ps, eps_sb, segs):
    """Per token segment: a = 1/sqrt(var+eps), b = -mu*a, then broadcast bf16
    copies of (a, b) to all 128 partitions (Pool)."""
    for off, w in segs:
        s5 = slice(off, off + w)
        nc.scalar.activation(t["tB"][:, s5], sum_ps[:, s5], AF.Square,
                             scale=1.0 / D)
        nc.vector.tensor_scalar_mul(t["mu"][:, s5], sum_ps[:, s5], -1.0 / D)
        nc.vector.scalar_tensor_tensor(
            out=t["var"][:, s5], in0=sq_ps[:, s5], scalar=1.0 / D,
            in1=t["tB"][:, s5], op0=ALU.mult, op1=ALU.subtract,
        )
        nc.scalar.activation(t["tB"][:, s5], t["var"][:, s5], AF.Sqrt,
                             bias=eps_sb, scale=1.0)
        nc.vector.reciprocal(t["a"][:, s5], t["tB"][:, s5])
        nc.vector.tensor_mul(t["b"][:, s5], t["mu"][:, s5], t["a"][:, s5])
        nc.scalar.copy(t["abf"][:, s5], t["a"][:, s5])
        nc.scalar.copy(t["bbf"][:, s5], t["b"][:, s5])
        nc.gpsimd.partition_broadcast(t["bcA"][:, s5], t["abf"][:, s5])
        nc.gpsimd.partition_broadcast(t["bcB"][:, s5], t["bbf"][:, s5])


@functools.lru_cache(maxsize=1)
def _build_program():
    nc = bacc.Bacc()

    def dp(name, shape, out=False, dt=f32):
        return nc.declare_dram_parameter(name, list(shape), dt, isOutput=out)

    # fp8 activations/weights, c-chunk index split as c = 2*cp + i
    x8q_d = dp("x8q", [P, 2, 2, TOK], dt=f8)
    x8b_d = dp("x8b", [P, 2, 2, S], dt=f8)
    xq_d = dp("xq", [P, DC, TOK], dt=bf16)      # residual spine input
    wq8_d = dp("wq8", [P, 2, 2, D], dt=f8)      # Wq/sqrt2
    wk8_d = dp("wk8", [P, 2, 2, D], dt=f8)      # Wk/sqrt2
    wv8_d = dp("wv8", [P, 2, 2, D], dt=f8)
    wo8_d = dp("wo8", [P, 2, 2, D], dt=f8)
    w18_d = dp("w18", [P, 2, 2, F], dt=f8)      # 16*g1*W1
    w2_d = dp("w2", [P, FC, D], dt=bf16)
    bqkvT_d = dp("bqkvT", [P, 12])              # q,k quarters pre-scaled
    boT_d = dp("boT", [P, DC])
    b1pT_d = dp("b1pT", [P, FC])
    b2pT_d = dp("b2pT", [P, DC])
    bvrow_d = dp("bvrow", [1, D])
    ones_col_d = dp("ones_col", [P, 1], dt=bf16)
    g1T_d = dp("g1T", [P, DC])
    g2T_d = dp("g2T", [P, DC])
    beta2T_d = dp("beta2T", [P, DC])
    outT_d = dp("outT", [P, DC, TOK], out=True, dt=bf16)

    with _TC(nc) as tc, ExitStack() as top:
        top.enter_context(
            nc.allow_low_precision(reason="fp8/bf16 matmul pipeline by design")
        )
        persist = top.enter_context(tc.tile_pool(name="persist", bufs=1))
        bqkvT_sb = persist.tile([P, 12], f32)
        boT_sb = persist.tile([P, DC], f32)
        b1pT_sb = persist.tile([P, FC], f32)
        b2pT_sb = persist.tile([P, DC], f32)
        g1T_sb = persist.tile([P, DC], f32)
        g2T_sb = persist.tile([P, DC], f32)
        beta2T_sb = persist.tile([P, DC], f32)
        bvb_sb = persist.tile([P, D], f32)
        ones128 = persist.tile([P, 1], bf16)
        eps_sb = persist.tile([1, 1], f32)
        nc.vector.memset(eps_sb, EPS)

        # weights + x (whole kernel lifetime)
        wP = top.enter_context(tc.tile_pool(name="wP", bufs=1))
        x8q_sb = wP.tile([P, 2, 2, TOK], f8)
        x8b_sb = wP.tile([P, 2, 2, S], f8)
        xq_sb = wP.tile([P, DC, TOK], bf16)
        wq8_sb = wP.tile([P, 2, 2, D], f8)
        wk8_sb = wP.tile([P, 2, 2, D], f8)
        wv8_sb = wP.tile([P, 2, 2, D], f8)
        wo8_sb = wP.tile([P, 2, 2, D], f8)
        w18_sb = wP.tile([P, 2, 2, F], f8)
        w2_sb = wP.tile([P, FC, D], bf16)

        # survives into the post phase
        mid = top.enter_context(tc.tile_pool(name="mid", bufs=1))
        ctxT8_sb = mid.tile([P, 2, 2, TOK], f8)   # [d-part, cp, i, tok]
        spine_sb = mid.tile([P, DC, TOK], f32)    # res1, later res2

        # ---- DMA queue: need order (biases first: they gate Q8/K8 writes) ----
        nc.sync.dma_start(out=bqkvT_sb, in_=bqkvT_d[:])
        nc.sync.dma_start(out=wq8_sb, in_=wq8_d[:])
        nc.sync.dma_start(out=x8q_sb, in_=x8q_d[:])
        nc.sync.dma_start(out=wk8_sb, in_=wk8_d[:])
        nc.sync.dma_start(out=x8b_sb[:, :, :, 0:512], in_=x8b_d[:, :, :, 0:512])
        nc.sync.dma_start(out=wv8_sb, in_=wv8_d[:])
        nc.gpsimd.dma_start(out=bvb_sb, in_=_bcast_ap(bvrow_d[:], P))
        nc.sync.dma_start(out=ones128, in_=ones_col_d[:])
        for q in range(1, 4):
            nc.sync.dma_start(out=x8b_sb[:, :, :, ts(q, 512)],
                              in_=x8b_d[:, :, :, ts(q, 512)])
        nc.sync.dma_start(out=boT_sb, in_=boT_d[:])
        nc.sync.dma_start(out=b1pT_sb, in_=b1pT_d[:])
        nc.sync.dma_start(out=b2pT_sb, in_=b2pT_d[:])
        nc.sync.dma_start(out=g1T_sb, in_=g1T_d[:])
        nc.sync.dma_start(out=g2T_sb, in_=g2T_d[:])
        nc.sync.dma_start(out=beta2T_sb, in_=beta2T_d[:])
        nc.sync.dma_start(out=wo8_sb, in_=wo8_d[:])
        nc.sync.dma_start(out=xq_sb, in_=xq_d[:])
        nc.sync.dma_start(out=w18_sb, in_=w18_d[:])
        for c in range(0, FC, 8):
            nc.sync.dma_start(out=w2_sb[:, c : c + 8, :],
                              in_=w2_d[:, c : c + 8, :])

        with ExitStack() as attn_scope:
            attnP = attn_scope.enter_context(tc.tile_pool(name="attnP", bufs=1))
            Q8_sb = attnP.tile([P, 2, 4, TOK], f8)    # [64(h%2)+hd, pl, j, tok]
            K8_sb = attnP.tile([P, 2, 4, S], f8)
            V8_sb = attnP.tile([P, KCP, 2, H, VW], f8)
            nc.gpsimd.memset(V8_sb[:, :, :, :, HD:VW], 1.0)

            psFill = attn_scope.enter_context(
                tc.tile_pool(name="fill_ps", bufs=2, space="PSUM"))
            psSc = attn_scope.enter_context(
                tc.tile_pool(name="sc_ps", bufs=2, space="PSUM"))
            psCtx = attn_scope.enter_context(
                tc.tile_pool(name="ctx_ps", bufs=1, space="PSUM"))
            expP = attn_scope.enter_context(tc.tile_pool(name="expP", bufs=3))
            nrmP = attn_scope.enter_context(tc.tile_pool(name="nrmP", bufs=2))

            def emit_q(j, th):
                s5 = ts(th, 512)
                q_ps = psFill.tile([P, 512], f32, name="q_ps", tag="fill")
                for cp in range(2):
                    nc.tensor.matmul(
                        q_ps,
                        lhsT=wq8_sb[:, cp, :, ts(j, P)],
                        rhs=x8q_sb[:, cp, :, s5],
                        start=(cp == 0), stop=(cp == 1),
                        perf_mode=DR, skip_group_check=True,
                    )
                nc.vector.tensor_scalar_add(
                    Q8_sb[:, 0, j, s5], q_ps, bqkvT_sb[:, j : j + 1])
                nc.gpsimd.tensor_copy(Q8_sb[:, 1, j, s5], Q8_sb[:, 0, j, s5])

            def emit_k(j, q):
                s5 = ts(q, 512)
                k_ps = psFill.tile([P, 512], f32, name="k_ps", tag="fill")
                for cp in range(2):
                    nc.tensor.matmul(
                        k_ps,
                        lhsT=wk8_sb[:, cp, :, ts(j, P)],
                        rhs=x8b_sb[:, cp, :, s5],
                        start=(cp == 0), stop=(cp == 1),
                        perf_mode=DR, skip_group_check=True,
                    )
                nc.vector.tensor_scalar_add(
                    K8_sb[:, 0, j, s5], k_ps, bqkvT_sb[:, 4 + j : 5 + j])
                nc.gpsimd.tensor_copy(K8_sb[:, 1, j, s5], K8_sb[:, 0, j, s5])

            bvb_h = bvb_sb.rearrange("p (h e) -> p h e", e=HD)

            def emit_v(kc):
                v_ps = psFill.tile([P, D], f32, name="v_ps", tag="fill")
                for cp in range(2):
                    nc.tensor.matmul(
                        v_ps,
                        lhsT=x8b_sb[:, cp, :, ts(kc, P)],
                        rhs=wv8_sb[:, cp, :, :],
                        start=(cp == 0), stop=(cp == 1),
                        perf_mode=DR, skip_group_check=True,
                    )
                nc.vector.tensor_tensor(
                    V8_sb[:, kc // 2, kc % 2, :, 0:HD],
                    v_ps.rearrange("p (h e) -> p h e", e=HD),
                    bvb_h, op=ALU.add,
                )

            # fill schedule: (h, kc) -> list of closures
            fill = {}
            fill[(0, 1)] = [lambda: emit_k(0, 1)]
            fill[(0, 5)] = [lambda: emit_k(0, 2)]
            fill[(0, 9)] = [lambda: emit_k(0, 3)]
            fill[(0, 13)] = [lambda: emit_q(1, 0)]
            fill[(0, 14)] = [lambda: emit_q(1, 1)]
            fill[(1, 1)] = [lambda: emit_k(1, 0)]
            fill[(1, 3)] = [lambda: emit_k(1, 1)]
            fill[(1, 5)] = [lambda: emit_k(1, 2)]
            fill[(1, 7)] = [lambda: emit_k(1, 3)]
            fill[(1, 9)] = [lambda: emit_q(2, 0)]
            fill[(1, 11)] = [lambda: emit_q(2, 1)]
            fill[(2, 3)] = [lambda: emit_k(2, 0)]
            fill[(2, 7)] = [lambda: emit_k(2, 1)]
            fill[(2, 11)] = [lambda: emit_k(2, 2)]
            fill[(2, 15)] = [lambda: emit_k(2, 3)]
            fill[(3, 5)] = [lambda: emit_q(3, 0)]
            fill[(3, 9)] = [lambda: emit_q(3, 1)]
            fill[(4, 3)] = [lambda: emit_k(3, 0)]
            fill[(4, 7)] = [lambda: emit_k(3, 1)]
            fill[(4, 11)] = [lambda: emit_k(3, 2)]
            fill[(4, 15)] = [lambda: emit_k(3, 3)]

            # upfront projections for head 0
            emit_q(0, 0)
            emit_q(0, 1)
            emit_k(0, 0)

            for h in range(H):
                j, hb = h // 2, 64 * (h % 2)
                ctx_ps = psCtx.tile([VW, TOK], f32, name="ctx_ps", tag="ctx")
                exps = []

                def emit_av(kcp_, e, h=h, ctx_ps=ctx_ps):
                    for th in range(2):
                        nc.tensor.matmul(
                            ctx_ps[:, ts(th, 512)],
                            lhsT=V8_sb[:, kcp_, :, h, :],
                            rhs=e[:, :, ts(th, 512)],
                            start=(kcp_ == 0), stop=(kcp_ == KCP - 1),
                            perf_mode=DR, skip_group_check=True,
                        )

                exp_t = None
                for kc in range(SC):
                    for f_ in fill.get((h, kc), ()):
                        f_()
                    if h == 0 and kc < 14:
                        emit_v(kc)
                    if kc % 2 == 0:
                        exp_t = expP.tile([P, 2, TOK], f8, name="exp8",
                                          tag="exp8")
                        exps.append(exp_t)
                    sc_ps = psSc.tile([P, TOK], f32, name="sc_ps", tag="sc")
                    for th in range(2):
                        nc.tensor.matmul(
                            sc_ps[:, ts(th, 512)],
                            lhsT=K8_sb[hb : hb + HD, :, j, ts(kc, P)],
                            rhs=Q8_sb[hb : hb + HD, :, j, ts(th, 512)],
                            start=True, stop=True,
                            perf_mode=DR, skip_group_check=True,
                        )
                    nc.scalar.activation(exp_t[:, kc % 2, :], sc_ps, AF.Exp,
                                         scale=0.125)
                    if kc % 2 == 1 and kc >= 3:
                        emit_av(kc // 2 - 1, exps[kc // 2 - 1])
                if h == 0:
                    emit_v(14)
                    emit_v(15)
                emit_av(KCP - 1, exps[KCP - 1])

                if h < H - 1:
                    # normalize off-psum: recip row 64, copy out, bcast, scale
                    rden = nrmP.tile([1, TOK], f32, name="rden", tag="rden")
                    nc.vector.reciprocal(rden, ctx_ps[HD : HD + 1, :])
                    ctmp = nrmP.tile([VW, TOK], f32, name="ctmp", tag="ctmp")
                    nc.vector.tensor_copy(ctmp, ctx_ps)
                    rb = nrmP.tile([HD, TOK], f32, name="rb", tag="rb")
                    nc.gpsimd.partition_broadcast(rb, rden)
                    nc.vector.tensor_tensor(
                        ctxT8_sb[hb : hb + HD, h // 4, (h // 2) % 2, :],
                        ctmp[0:HD, :], rb, op=ALU.mult,
                    )
                else:
                    # last head: quarter-granularity straight from PSUM so
                    # Wo(q0) starts ~1us after the last AV, not ~5us
                    for q in range(4):
                        sq_ = ts(q, 256)
                        rden = nrmP.tile([1, 256], f32, name="rdq", tag="rdq")
                        nc.vector.reciprocal(rden, ctx_ps[HD : HD + 1, sq_])
                        rb = nrmP.tile([HD, 256], f32, name="rbq", tag="rbq")
                        nc.gpsimd.partition_broadcast(rb, rden)
                        nc.vector.tensor_tensor(
                            ctxT8_sb[hb : hb + HD, h // 4, (h // 2) % 2, sq_],
                            ctx_ps[0:HD, sq_], rb, op=ALU.mult,
                        )

        # ---- post phase: Wo + LN1 + FFN1 (fp8 DR) + FFN2 (bf16) + LN2 ----
        postP = top.enter_context(tc.tile_pool(name="postP", bufs=1))
        ln18_sb = postP.tile([P, 2, 2, TOK], f8)
        ln1g_sb = postP.tile([P, DC, TOK], bf16)
        hid_sb = postP.tile([P, FC, TOK], bf16)
        out_sb = postP.tile([P, DC, TOK], bf16)
        workP = top.enter_context(tc.tile_pool(name="workP", bufs=2))
        ln1t = _ln_alloc(nc, postP, "ln1")
        ln2t = _ln_alloc(nc, postP, "ln2")

        TQ = 256  # post-phase pipeline granularity (tokens)

        def emit_stats(src_slice, sum_ps, sq_ps, s5, first, last, tagp):
            sbf = workP.tile([P, TQ], bf16, name=f"{tagp}_sbf",
                             tag=f"{tagp}_sbf")
            nc.gpsimd.tensor_copy(sbf, src_slice)
            sq = workP.tile([P, TQ], bf16, name=f"{tagp}_sq", tag=f"{tagp}_sq")
            nc.vector.tensor_mul(sq, sbf, sbf)
            nc.tensor.matmul(sum_ps[:, s5], lhsT=ones128, rhs=sbf,
                             start=first, stop=last, skip_group_check=True)
            nc.tensor.matmul(sq_ps[:, s5], lhsT=ones128, rhs=sq,
                             start=first, stop=last, skip_group_check=True)

        # Wo (fp8 DR) + LN1, quarter-granularity pipeline
        with tc.tile_pool(name="wo_ps", bufs=3, space="PSUM") as psWo, \
             tc.tile_pool(name="ln1s_ps", bufs=1, space="PSUM") as psS1:
            sum1_ps = psS1.tile([1, TOK], f32, name="ln1_sum")
            sq1_ps = psS1.tile([1, TOK], f32, name="ln1_sqsum")
            for q in range(4):
                s5 = ts(q, TQ)
                for m in range(DC):
                    wo_ps = psWo.tile([P, TQ], f32, name="wo_ps", tag="wo")
                    for cp in range(2):
                        nc.tensor.matmul(
                            wo_ps,
                            lhsT=wo8_sb[:, cp, :, ts(m, P)],
                            rhs=ctxT8_sb[:, cp, :, s5],
                            start=(cp == 0), stop=(cp == 1),
                            perf_mode=DR, skip_group_check=True,
                        )
                    nc.vector.scalar_tensor_tensor(
                        out=spine_sb[:, m, s5], in0=wo_ps,
                        scalar=boT_sb[:, m : m + 1], in1=xq_sb[:, m, s5],
                        op0=ALU.add, op1=ALU.add,
                    )
                    emit_stats(spine_sb[:, m, s5], sum1_ps, sq1_ps, s5,
                               m == 0, m == DC - 1, "s1")
                _ln_chain(nc, ln1t, sum1_ps, sq1_ps, eps_sb, [(q * TQ, TQ)])
                # combine: ln18 (f8, FFN1 input) + ln1g (bf16, FFN2 residual)
                for c in range(DC):
                    v = workP.tile([P, TQ], bf16, name="ln1v", tag="ln1v")
                    nc.vector.tensor_mul(v, spine_sb[:, c, s5],
                                         ln1t["bcA"][:, s5])
                    t = workP.tile([P, TQ], bf16, name="ln1t", tag="ln1t")
                    nc.vector.tensor_tensor(t, v, ln1t["bcB"][:, s5],
                                            op=ALU.add)
                    nc.gpsimd.tensor_copy(ln18_sb[:, c // 2, c % 2, s5], t)
                    nc.vector.tensor_scalar_mul(
                        ln1g_sb[:, c, s5], t, g1T_sb[:, c : c + 1])

        # FFN1 (fp8 DR) + relu (alternating ACT/DVE; hid is 16x-scaled,
        # compensated by W2/16 host-side)
        with tc.tile_pool(name="f1_ps", bufs=3, space="PSUM") as psF1:
            for q in range(4):
                s5 = ts(q, TQ)
                for m in range(FC):
                    h_ps = psF1.tile([P, TQ], f32, name="h_ps", tag="h")
                    for cp in range(2):
                        nc.tensor.matmul(
                            h_ps,
                            lhsT=w18_sb[:, cp, :, ts(m, P)],
                            rhs=ln18_sb[:, cp, :, s5],
                            start=(cp == 0), stop=(cp == 1),
                            perf_mode=DR, skip_group_check=True,
                        )
                    if m % 2 == 0:
                        nc.scalar.activation(
                            hid_sb[:, m, s5], h_ps, AF.Relu,
                            bias=b1pT_sb[:, m : m + 1], scale=1.0,
                        )
                    else:
                        nc.vector.tensor_scalar(
                            hid_sb[:, m, s5], h_ps, b1pT_sb[:, m : m + 1],
                            0.0, ALU.add, ALU.max)

        # FFN2 (bf16) + LN2, quarter-granularity pipeline
        with tc.tile_pool(name="f2_ps", bufs=3, space="PSUM") as psF2, \
             tc.tile_pool(name="ln2s_ps", bufs=1, space="PSUM") as psS2:
            sum2_ps = psS2.tile([1, TOK], f32, name="ln2_sum")
            sq2_ps = psS2.tile([1, TOK], f32, name="ln2_sqsum")
            for q in range(4):
                s5 = ts(q, TQ)
                for m in range(DC):
                    f_ps = psF2.tile([P, TQ], f32, name="f_ps", tag="f")
                    for c in range(FC):
                        nc.tensor.matmul(
                            f_ps,
                            lhsT=w2_sb[:, c, ts(m, P)],
                            rhs=hid_sb[:, c, s5],
                            start=(c == 0), stop=(c == FC - 1),
                            skip_group_check=True,
                        )
                    nc.vector.scalar_tensor_tensor(
                        out=spine_sb[:, m, s5], in0=f_ps,
                        scalar=b2pT_sb[:, m : m + 1], in1=ln1g_sb[:, m, s5],
                        op0=ALU.add, op1=ALU.add,
                    )
                    emit_stats(spine_sb[:, m, s5], sum2_ps, sq2_ps, s5,
                               m == 0, m == DC - 1, "s2")
                _ln_chain(nc, ln2t, sum2_ps, sq2_ps, eps_sb, [(q * TQ, TQ)])
                for c in range(DC):
                    v = workP.tile([P, TQ], bf16, name="ln2v", tag="ln2v")
                    nc.vector.tensor_mul(v, spine_sb[:, c, s5],
                                         ln2t["bcA"][:, s5])
                    t = workP.tile([P, TQ], bf16, name="ln2t", tag="ln2t")
                    nc.vector.tensor_tensor(t, v, ln2t["bcB"][:, s5],
                                            op=ALU.add)
                    eng = nc.vector if c % 2 == 0 else nc.gpsimd
                    eng.tensor_scalar(
                        out_sb[:, c, s5], t, g2T_sb[:, c : c + 1],
                        beta2T_sb[:, c : c + 1], ALU.mult, ALU.add)
                    nc.sync.dma_start(out=outT_d[:, c, s5],
                                      in_=out_sb[:, c, s5])

    if not nc.is_finalized():
        nc.finalize()
    return nc


def _prep_inputs(x, Wqkv, bqkv, Wo, bo, g1, beta1, W1, b1, W2, b2, g2, beta2):
    """Host-side sharding/layout prep -> list of 8 in_maps."""
    f = lambda a: np.ascontiguousarray(np.asarray(a, dtype=np.float32))
    bf = lambda a: np.ascontiguousarray(
        np.asarray(a, dtype=np.float32).astype(ml_dtypes.bfloat16))
    q8 = lambda a: np.ascontiguousarray(
        np.asarray(a, dtype=np.float32).astype(F8NP))

    def pack8(w):  # [512, N] -> [128, 2, 2, N] fp8, c = 2*cp + i
        w = np.asarray(w, dtype=np.float32)
        return q8(w.reshape(2, 2, P, w.shape[1]).transpose(2, 0, 1, 3))

    def chunkT(w, nchunk, cast):  # [n*128, cols] -> [128, n, cols]
        w = np.asarray(w, dtype=np.float32)
        return cast(w.reshape(nchunk, P, w.shape[1]).transpose(1, 0, 2))

    Wqkv = np.asarray(Wqkv, np.float32)
    s2 = 1.0 / np.sqrt(2.0)
    bqkv_s = np.asarray(bqkv, np.float32).copy()
    bqkv_s[: 2 * D] *= s2                      # q,k bias pre-scaled
    b1p = np.asarray(b1, np.float32) + np.asarray(beta1, np.float32) @ np.asarray(W1, np.float32)
    b2p = np.asarray(b2, np.float32) + np.asarray(beta1, np.float32)
    shared = {
        "wq8": pack8(Wqkv[:, 0:D] * s2),
        "wk8": pack8(Wqkv[:, D : 2 * D] * s2),
        "wv8": pack8(Wqkv[:, 2 * D :]),
        "wo8": pack8(Wo),
        "w18": pack8(np.asarray(W1, np.float32)
                     * np.asarray(g1, np.float32)[:, None] * 16.0),
        "w2": chunkT(np.asarray(W2, np.float32) / 16.0, FC, bf),
        "bqkvT": f(bqkv_s.reshape(12, P).T),
        "boT": f(np.asarray(bo).reshape(DC, P).T),
        "b1pT": f(b1p.reshape(FC, P).T * 16.0),
        "b2pT": f(b2p.reshape(DC, P).T),
        "bvrow": f(np.asarray(bqkv, np.float32)[2 * D :].reshape(1, D)),
        "ones_col": np.ones((P, 1), ml_dtypes.bfloat16),
        "g1T": f(np.asarray(g1).reshape(DC, P).T),
        "g2T": f(np.asarray(g2).reshape(DC, P).T),
        "beta2T": f(np.asarray(beta2).reshape(DC, P).T),
    }
    x = np.asarray(x, dtype=np.float32)
    in_maps = []
    for c in range(8):
        b, half = c // 2, c % 2
        xbT = x[b].T.reshape(2, 2, P, S).transpose(2, 0, 1, 3)   # [128,2,2,S]
        xq = x[b, half * TOK : (half + 1) * TOK]
        xqT4 = xq.T.reshape(DC, P, TOK).transpose(1, 0, 2)        # [128,4,TOK]
        x8qT = xq.T.reshape(2, 2, P, TOK).transpose(2, 0, 1, 3)
        in_maps.append(dict(
            shared, x8b=q8(xbT), x8q=q8(x8qT), xq=bf(xqT4)))
    return in_maps


def kernel(**inputs):
    from concourse.bass_utils import run_bass_kernel_spmd

    nc = _build_program()
    in_maps = _prep_inputs(**inputs)
    res = run_bass_kernel_spmd(nc, in_maps, core_ids=list(range(8)))
    out = np.empty((B, S, D), dtype=np.float32)
    for c in range(8):
        b, half = c // 2, c % 2
        oT = np.asarray(res.results[c]["outT"], dtype=np.float32)  # [P,DC,TOK]
        out[b, half * TOK : (half + 1) * TOK] = (
            oT.transpose(2, 1, 0).reshape(TOK, D)
        )
    return out


# revision 11
# speedup vs baseline: 1.0065x; 1.0065x over previous
"""Trainium2 Bass kernel for a dense transformer encoder layer.

Shapes: B=4, S=2048, D=512, H=8 heads (HD=64), FFN F=2048.

Sharding (8 NeuronCores, no collectives): core c handles batch b = c//2 and
query-half half = c%2 (1024 query tokens); K/V are computed for the full
2048-token sequence on both cores of a pair.

Design notes:
- The attention phase is ACT-bound: the 128 softmax exp ops ([128,1024] each)
  set a ~135us floor. Everything else is scheduled under that roof.
- All attention-path matmuls (QKV proj, scores, AV, Wo) and FFN1 run as fp8e4
  DoubleRow matmuls (2 contraction tiles per instruction, 0.5 cycles/row).
  Scores use a duplicated-plane trick (both planes hold K/sqrt2, Q/sqrt2) to
  fill the 2x64 contraction; projections pair genuine 128-deep c-chunks.
- AV keeps exp stationary-free: lhsT = V8 [key,2,65] over kc pairs, rhs = the
  exp pair [128,2,1024]; the softmax denominator rides in V column 64.
- The residual spine (res1/res2) is fp32 to buy error headroom for fp8;
  LayerNorm stats run off bf16 copies via ones-matmuls.
- FFN2 stays bf16 (fp8 there costs too much accuracy).
- Host transposes x / un-transposes the output.
"""

import functools
import numpy as np
from contextlib import ExitStack

import ml_dtypes

import concourse.bass as bass
import concourse.tile as tile
import concourse.mybir as mybir
from concourse import bacc
from concourse.bass import ts
from concourse.vector_clock import ScopedClock

B, S, D, H, F = 4, 2048, 512, 8, 2048
HD = D // H           # 64
P = 128
DC = D // P           # 4   d chunks
FC = F // P           # 16  ffn chunks
SC = S // P           # 16  key chunks
KCP = SC // 2         # 8   key-chunk pairs
TOK = S // 2          # 1024 query tokens per core
EPS = 1e-5
VW = HD + 2           # 66: V cols per head incl. ones + pad (DR needs even)

f32 = mybir.dt.float32
bf16 = mybir.dt.bfloat16
f8 = mybir.dt.float8e4
AF = mybir.ActivationFunctionType
ALU = mybir.AluOpType
DR = mybir.MatmulPerfMode.DoubleRow

F8NP = ml_dtypes.float8_e4m3


class _TC(tile.TileContext):
    """TileContext whose tail drain splits sem waits one-per-drain: the
    walrus build in this container rejects >1 sync wait on an SP TPB_CTRL."""

    def _drain_and_barrier(self, tick_clock, wait_clock):
        nc = self.nc
        drain_inst = nc.sync.drain()
        wait_clock.add_sem_waits(
            drain_inst.ins, ScopedClock({None: tick_clock.global_clock})
        )
        si = drain_inst.ins.sync_info
        waits = list(si.on_wait) if si and si.on_wait else []
        MAXW = 1
        if len(waits) > MAXW:
            si.on_wait = waits[:MAXW]
            for i in range(MAXW, len(waits), MAXW):
                extra = nc.sync.drain()
                extra.ins.sync_info = mybir.SyncInfo(
                    on_wait=waits[i : i + MAXW], on_update=[]
                )
        nc.all_engine_barrier()
        popped = nc._tile_sem_poison_stack.pop()
        assert popped is self._sem_poison
        nc.clear_and_free_semaphores(list(self.sems.allocated().values()))
        nc.all_engine_barrier()


def _bcast_ap(row_ap, nparts):
    """AP reading the single-partition row `row_ap` broadcast to nparts."""
    return bass.AP(
        tensor=row_ap.tensor,
        offset=row_ap.offset,
        ap=[[0, nparts]] + [list(d) for d in row_ap.ap[1:]],
    )


def _ln_alloc(nc, stat_pool, tagp):
    return {
        "mu": stat_pool.tile([1, TOK], f32, name=f"{tagp}_mu", tag=f"{tagp}_mu"),
        "tB": stat_pool.tile([1, TOK], f32, name=f"{tagp}_tB", tag=f"{tagp}_tB"),
        "var": stat_pool.tile([1, TOK], f32, name=f"{tagp}_var", tag=f"{tagp}_var"),
        "a": stat_pool.tile([1, TOK], f32, name=f"{tagp}_a", tag=f"{tagp}_a"),
        "b": stat_pool.tile([1, TOK], f32, name=f"{tagp}_b", tag=f"{tagp}_b"),
        "abf": stat_pool.tile([1, TOK], bf16, name=f"{tagp}_abf",
                              tag=f"{tagp}_abf"),
        "bbf": stat_pool.tile([1, TOK], bf16, name=f"{tagp}_bbf",
                              tag=f"{tagp}_bbf"),
        "bcA": stat_pool.tile([P, TOK], bf16, name=f"{tagp}_bcA",
                              tag=f"{tagp}_bcA"),
        "bcB": stat_pool.tile([P, TOK], bf16, name=f"{tagp}_bcB",
                              tag=f"{tagp}_bcB"),
    }


def _ln_chain(nc, t, sum_ps, sq_ps, eps_sb, segs):
    """Per token segment: a = 1/sqrt(var+eps), b = -mu*a, then broadcast bf16
    copies of (a, b) to all 128 partitions (Pool)."""
    for off, w in segs:
        s5 = slice(off, off + w)
        nc.scalar.activation(t["tB"][:, s5], sum_ps[:, s5], AF.Square,
                             scale=1.0 / D)
        nc.vector.tensor_scalar_mul(t["mu"][:, s5], sum_ps[:, s5], -1.0 / D)
        nc.vector.scalar_tensor_tensor(
            out=t["var"][:, s5], in0=sq_ps[:, s5], scalar=1.0 / D,
            in1=t["tB"][:, s5], op0=ALU.mult, op1=ALU.subtract,
        )
        nc.scalar.activation(t["tB"][:, s5], t["var"][:, s5], AF.Sqrt,
                             bias=eps_sb, scale=1.0)
        nc.vector.reciprocal(t["a"][:, s5], t["tB"][:, s5])
        nc.vector.tensor_mul(t["b"][:, s5], t["mu"][:, s5], t["a"][:, s5])
        nc.scalar.copy(t["abf"][:, s5], t["a"][:, s5])
        nc.scalar.copy(t["bbf"][:, s5], t["b"][:, s5])
        nc.gpsimd.partition_broadcast(t["bcA"][:, s5], t["abf"][:, s5])
        nc.gpsimd.partition_broadcast(t["bcB"][:, s5], t["bbf"][:, s5])


@functools.lru_cache(maxsize=1)
def _build_program():
    nc = bacc.Bacc()

    def dp(name, shape, out=False, dt=f32):
        return nc.declare_dram_parameter(name, list(shape), dt, isOutput=out)

    # fp8 activations/weights, c-chunk index split as c = 2*cp + i
    x8q_d = dp("x8q", [P, 2, 2, TOK], dt=f8)
    x8b_d = dp("x8b", [P, 2, 2, S], dt=f8)
    xq_d = dp("xq", [P, DC, TOK], dt=bf16)      # residual spine input
    wq8_d = dp("wq8", [P, 2, 2, D], dt=f8)      # Wq/sqrt2
    wk8_d = dp("wk8", [P, 2, 2, D], dt=f8)      # Wk/sqrt2
    wv8_d = dp("wv8", [P, 2, 2, D], dt=f8)
    wo8_d = dp("wo8", [P, 2, 2, D], dt=f8)
    w18_d = dp("w18", [P, 2, 2, F], dt=f8)      # 16*g1*W1
    w2_d = dp("w2", [P, FC, D], dt=bf16)
    bqkvT_d = dp("bqkvT", [P, 12])              # q,k quarters pre-scaled
    boT_d = dp("boT", [P, DC])
    b1pT_d = dp("b1pT", [P, FC])
    b2pT_d = dp("b2pT", [P, DC])
    bvrow_d = dp("bvrow", [1, D])
    ones_col_d = dp("ones_col", [P, 1], dt=bf16)
    g1T_d = dp("g1T", [P, DC])
    g2T_d = dp("g2T", [P, DC])
    beta2T_d = dp("beta2T", [P, DC])
    outT_d = dp("outT", [P, DC, TOK], out=True, dt=bf16)

    with _TC(nc) as tc, ExitStack() as top:
        top.enter_context(
            nc.allow_low_precision(reason="fp8/bf16 matmul pipeline by design")
        )
        persist = top.enter_context(tc.tile_pool(name="persist", bufs=1))
        bqkvT_sb = persist.tile([P, 12], f32)
        boT_sb = persist.tile([P, DC], f32)
        b1pT_sb = persist.tile([P, FC], f32)
        b2pT_sb = persist.tile([P, DC], f32)
        g1T_sb = persist.tile([P, DC], f32)
        g2T_sb = persist.tile([P, DC], f32)
        beta2T_sb = persist.tile([P, DC], f32)
        bvb_sb = persist.tile([P, D], f32)
        ones128 = persist.tile([P, 1], bf16)
        eps_sb = persist.tile([1, 1], f32)
        nc.vector.memset(eps_sb, EPS)

        # weights + x (whole kernel lifetime)
        wP = top.enter_context(tc.tile_pool(name="wP", bufs=1))
        x8q_sb = wP.tile([P, 2, 2, TOK], f8)
        x8b_sb = wP.tile([P, 2, 2, S], f8)
        xq_sb = wP.tile([P, DC, TOK], bf16)
        wq8_sb = wP.tile([P, 2, 2, D], f8)
        wk8_sb = wP.tile([P, 2, 2, D], f8)
        wv8_sb = wP.tile([P, 2, 2, D], f8)
        wo8_sb = wP.tile([P, 2, 2, D], f8)
        w18_sb = wP.tile([P, 2, 2, F], f8)
        w2_sb = wP.tile([P, FC, D], bf16)

        # survives into the post phase
        mid = top.enter_context(tc.tile_pool(name="mid", bufs=1))
        ctxT8_sb = mid.tile([P, 2, 2, TOK], f8)   # [d-part, cp, i, tok]
        spine_sb = mid.tile([P, DC, TOK], f32)    # res1, later res2

        # ---- DMA queue: need order (biases first: they gate Q8/K8 writes) ----
        nc.sync.dma_start(out=bqkvT_sb, in_=bqkvT_d[:])
        nc.sync.dma_start(out=wq8_sb, in_=wq8_d[:])
        nc.sync.dma_start(out=x8q_sb, in_=x8q_d[:])
        nc.sync.dma_start(out=wk8_sb, in_=wk8_d[:])
        nc.sync.dma_start(out=x8b_sb[:, :, :, 0:512], in_=x8b_d[:, :, :, 0:512])
        nc.sync.dma_start(out=wv8_sb, in_=wv8_d[:])
        nc.gpsimd.dma_start(out=bvb_sb, in_=_bcast_ap(bvrow_d[:], P))
        nc.sync.dma_start(out=ones128, in_=ones_col_d[:])
        for q in range(1, 4):
            nc.sync.dma_start(out=x8b_sb[:, :, :, ts(q, 512)],
                              in_=x8b_d[:, :, :, ts(q, 512)])
        nc.sync.dma_start(out=boT_sb, in_=boT_d[:])
        nc.sync.dma_start(out=b1pT_sb, in_=b1pT_d[:])
        nc.sync.dma_start(out=b2pT_sb, in_=b2pT_d[:])
        nc.sync.dma_start(out=g1T_sb, in_=g1T_d[:])
        nc.sync.dma_start(out=g2T_sb, in_=g2T_d[:])
        nc.sync.dma_start(out=beta2T_sb, in_=beta2T_d[:])
        nc.sync.dma_start(out=wo8_sb, in_=wo8_d[:])
        nc.sync.dma_start(out=xq_sb, in_=xq_d[:])
        nc.sync.dma_start(out=w18_sb, in_=w18_d[:])
        for c in range(0, FC, 8):
            nc.sync.dma_start(out=w2_sb[:, c : c + 8, :],
                              in_=w2_d[:, c : c + 8, :])

        with ExitStack() as attn_scope:
            attnP = attn_scope.enter_context(tc.tile_pool(name="attnP", bufs=1))
            Q8_sb = attnP.tile([P, 2, 4, TOK], f8)    # [64(h%2)+hd, pl, j, tok]
            K8_sb = attnP.tile([P, 2, 4, S], f8)
            V8_sb = attnP.tile([P, KCP, 2, H, VW], f8)
            nc.gpsimd.memset(V8_sb[:, :, :, :, HD:VW], 1.0)

            psFill = attn_scope.enter_context(
                tc.tile_pool(name="fill_ps", bufs=2, space="PSUM"))
            psSc = attn_scope.enter_context(
                tc.tile_pool(name="sc_ps", bufs=2, space="PSUM"))
            psCtx = attn_scope.enter_context(
                tc.tile_pool(name="ctx_ps", bufs=1, space="PSUM"))
            expP = attn_scope.enter_context(tc.tile_pool(name="expP", bufs=3))
            nrmP = attn_scope.enter_context(tc.tile_pool(name="nrmP", bufs=2))

            def emit_q(j, th):
                s5 = ts(th, 512)
                q_ps = psFill.tile([P, 512], f32, name="q_ps", tag="fill")
                for cp in range(2):
                    nc.tensor.matmul(
                        q_ps,
                        lhsT=wq8_sb[:, cp, :, ts(j, P)],
                        rhs=x8q_sb[:, cp, :, s5],
                        start=(cp == 0), stop=(cp == 1),
                        perf_mode=DR, skip_group_check=True,
                    )
                nc.vector.tensor_scalar_add(
                    Q8_sb[:, 0, j, s5], q_ps, bqkvT_sb[:, j : j + 1])
                nc.gpsimd.tensor_copy(Q8_sb[:, 1, j, s5], Q8_sb[:, 0, j, s5])

            def emit_k(j, q):
                s5 = ts(q, 512)
                k_ps = psFill.tile([P, 512], f32, name="k_ps", tag="fill")
                for cp in range(2):
                    nc.tensor.matmul(
                        k_ps,
                        lhsT=wk8_sb[:, cp, :, ts(j, P)],
                        rhs=x8b_sb[:, cp, :, s5],
                        start=(cp == 0), stop=(cp == 1),
                        perf_mode=DR, skip_group_check=True,
                    )
                nc.vector.tensor_scalar_add(
                    K8_sb[:, 0, j, s5], k_ps, bqkvT_sb[:, 4 + j : 5 + j])
                nc.gpsimd.tensor_copy(K8_sb[:, 1, j, s5], K8_sb[:, 0, j, s5])

            bvb_h = bvb_sb.rearrange("p (h e) -> p h e", e=HD)

            def emit_v(kc):
                v_ps = psFill.tile([P, D], f32, name="v_ps", tag="fill")
                for cp in range(2):
                    nc.tensor.matmul(
                        v_ps,
                        lhsT=x8b_sb[:, cp, :, ts(kc, P)],
                        rhs=wv8_sb[:, cp, :, :],
                        start=(cp == 0), stop=(cp == 1),
                        perf_mode=DR, skip_group_check=True,
                    )
                nc.vector.tensor_tensor(
                    V8_sb[:, kc // 2, kc % 2, :, 0:HD],
                    v_ps.rearrange("p (h e) -> p h e", e=HD),
                    bvb_h, op=ALU.add,
                )

            # fill schedule: (h, kc) -> list of closures
            fill = {}
            fill[(0, 1)] = [lambda: emit_k(0, 1)]
            fill[(0, 5)] = [lambda: emit_k(0, 2)]
            fill[(0, 9)] = [lambda: emit_k(0, 3)]
            fill[(0, 13)] = [lambda: emit_q(1, 0)]
            fill[(0, 14)] = [lambda: emit_q(1, 1)]
            fill[(1, 1)] = [lambda: emit_k(1, 0)]
            fill[(1, 3)] = [lambda: emit_k(1, 1)]
            fill[(1, 5)] = [lambda: emit_k(1, 2)]
            fill[(1, 7)] = [lambda: emit_k(1, 3)]
            fill[(1, 9)] = [lambda: emit_q(2, 0)]
            fill[(1, 11)] = [lambda: emit_q(2, 1)]
            fill[(2, 3)] = [lambda: emit_k(2, 0)]
            fill[(2, 7)] = [lambda: emit_k(2, 1)]
            fill[(2, 11)] = [lambda: emit_k(2, 2)]
            fill[(2, 15)] = [lambda: emit_k(2, 3)]
            fill[(3, 5)] = [lambda: emit_q(3, 0)]
            fill[(3, 9)] = [lambda: emit_q(3, 1)]
            fill[(4, 3)] = [lambda: emit_k(3, 0)]
            fill[(4, 7)] = [lambda: emit_k(3, 1)]
            fill[(4, 11)] = [lambda: emit_k(3, 2)]
            fill[(4, 15)] = [lambda: emit_k(3, 3)]

            # upfront projections for head 0
            emit_q(0, 0)
            emit_q(0, 1)
            emit_k(0, 0)

            for h in range(H):
                j, hb = h // 2, 64 * (h % 2)
                ctx_ps = psCtx.tile([VW, TOK], f32, name="ctx_ps", tag="ctx")
                exps = []

                def emit_av(kcp_, e, h=h, ctx_ps=ctx_ps):
                    for th in range(2):
                        nc.tensor.matmul(
                            ctx_ps[:, ts(th, 512)],
                            lhsT=V8_sb[:, kcp_, :, h, :],
                            rhs=e[:, :, ts(th, 512)],
                            start=(kcp_ == 0), stop=(kcp_ == KCP - 1),
                            perf_mode=DR, skip_group_check=True,
                        )

                exp_t = None
                for kc in range(SC):
                    for f_ in fill.get((h, kc), ()):
                        f_()
                    if h == 0 and kc < 14:
                        emit_v(kc)
                    if kc % 2 == 0:
                        exp_t = expP.tile([P, 2, TOK], f8, name="exp8",
                                          tag="exp8")
                        exps.append(exp_t)
                    sc_ps = psSc.tile([P, TOK], f32, name="sc_ps", tag="sc")
                    for th in range(2):
                        nc.tensor.matmul(
                            sc_ps[:, ts(th, 512)],
                            lhsT=K8_sb[hb : hb + HD, :, j, ts(kc, P)],
                            rhs=Q8_sb[hb : hb + HD, :, j, ts(th, 512)],
                            start=True, stop=True,
                            perf_mode=DR, skip_group_check=True,
                        )
                    nc.scalar.activation(exp_t[:, kc % 2, :], sc_ps, AF.Exp,
                                         scale=0.125)
                    if kc % 2 == 1 and kc >= 3:
                        emit_av(kc // 2 - 1, exps[kc // 2 - 1])
                if h == 0:
                    emit_v(14)
                    emit_v(15)
                emit_av(KCP - 1, exps[KCP - 1])

                if h < H - 1:
                    # normalize off-psum: recip row 64, copy out, bcast, scale
                    rden = nrmP.tile([1, TOK], f32, name="rden", tag="rden")
                    nc.vector.reciprocal(rden, ctx_ps[HD : HD + 1, :])
                    ctmp = nrmP.tile([VW, TOK], f32, name="ctmp", tag="ctmp")
                    nc.vector.tensor_copy(ctmp, ctx_ps)
                    rb = nrmP.tile([HD, TOK], f32, name="rb", tag="rb")
                    nc.gpsimd.partition_broadcast(rb, rden)
                    nc.vector.tensor_tensor(
                        ctxT8_sb[hb : hb + HD, h // 4, (h // 2) % 2, :],
                        ctmp[0:HD, :], rb, op=ALU.mult,
                    )
                else:
                    # last head: quarter-granularity straight from PSUM so
                    # Wo(q0) starts ~1us after the last AV, not ~5us
                    for q in range(4):
                        sq_ = ts(q, 256)
                        rden = nrmP.tile([1, 256], f32, name="rdq", tag="rdq")
                        nc.vector.reciprocal(rden, ctx_ps[HD : HD + 1, sq_])
                        rb = nrmP.tile([HD, 256], f32, name="rbq", tag="rbq")
                        nc.gpsimd.partition_broadcast(rb, rden)
                        nc.vector.tensor_tensor(
                            ctxT8_sb[hb : hb + HD, h // 4, (h // 2) % 2, sq_],
                            ctx_ps[0:HD, sq_], rb, op=ALU.mult,
                        )

        # ---- post phase: Wo + LN1 + FFN1 (fp8 DR) + FFN2 (bf16) + LN2 ----
        postP = top.enter_context(tc.tile_pool(name="postP", bufs=1))
        ln18_sb = postP.tile([P, 2, 2, TOK], f8)
        ln1g_sb = postP.tile([P, DC, TOK], bf16)
        hid_sb = postP.tile([P, FC, TOK], bf16)
        out_sb = postP.tile([P, DC, TOK], bf16)
        workP = top.enter_context(tc.tile_pool(name="workP", bufs=4))
        ln1t = _ln_alloc(nc, postP, "ln1")
        ln2t = _ln_alloc(nc, postP, "ln2")

        TQ = 256  # post-phase pipeline granularity (tokens)

        def emit_stats(src_slice, sum_ps, sq_ps, s5, first, last, tagp):
            sbf = workP.tile([P, TQ], bf16, name=f"{tagp}_sbf",
                             tag=f"{tagp}_sbf")
            nc.gpsimd.tensor_copy(sbf, src_slice)
            sq = workP.tile([P, TQ], bf16, name=f"{tagp}_sq", tag=f"{tagp}_sq")
            nc.vector.tensor_mul(sq, sbf, sbf)
            nc.tensor.matmul(sum_ps[:, s5], lhsT=ones128, rhs=sbf,
                             start=first, stop=last, skip_group_check=True)
            nc.tensor.matmul(sq_ps[:, s5], lhsT=ones128, rhs=sq,
                             start=first, stop=last, skip_group_check=True)

        # Wo (fp8 DR) + LN1, quarter-granularity pipeline
        with tc.tile_pool(name="wo_ps", bufs=4, space="PSUM") as psWo, \
             tc.tile_pool(name="ln1s_ps", bufs=1, space="PSUM") as psS1:
            sum1_ps = psS1.tile([1, TOK], f32, name="ln1_sum")
            sq1_ps = psS1.tile([1, TOK], f32, name="ln1_sqsum")
            for q in range(4):
                s5 = ts(q, TQ)
                for m in range(DC):
                    wo_ps = psWo.tile([P, TQ], f32, name="wo_ps", tag="wo")
                    for cp in range(2):
                        nc.tensor.matmul(
                            wo_ps,
                            lhsT=wo8_sb[:, cp, :, ts(m, P)],
                            rhs=ctxT8_sb[:, cp, :, s5],
                            start=(cp == 0), stop=(cp == 1),
                            perf_mode=DR, skip_group_check=True,
                        )
                    nc.vector.scalar_tensor_tensor(
                        out=spine_sb[:, m, s5], in0=wo_ps,
                        scalar=boT_sb[:, m : m + 1], in1=xq_sb[:, m, s5],
                        op0=ALU.add, op1=ALU.add,
                    )
                    emit_stats(spine_sb[:, m, s5], sum1_ps, sq1_ps, s5,
                               m == 0, m == DC - 1, "s1")
                _ln_chain(nc, ln1t, sum1_ps, sq1_ps, eps_sb, [(q * TQ, TQ)])
                # combine: ln18 (f8, FFN1 input) + ln1g (bf16, FFN2 residual)
                for c in range(DC):
                    v = workP.tile([P, TQ], bf16, name="ln1v", tag="ln1v")
                    nc.vector.tensor_mul(v, spine_sb[:, c, s5],
                                         ln1t["bcA"][:, s5])
                    t = workP.tile([P, TQ], bf16, name="ln1t", tag="ln1t")
                    nc.vector.tensor_tensor(t, v, ln1t["bcB"][:, s5],
                                            op=ALU.add)
                    nc.gpsimd.tensor_copy(ln18_sb[:, c // 2, c % 2, s5], t)
                    nc.gpsimd.tensor_scalar_mul(
                        ln1g_sb[:, c, s5], t, g1T_sb[:, c : c + 1])

        # FFN1 (fp8 DR) + relu (alternating ACT/DVE; hid is 16x-scaled,
        # compensated by W2/16 host-side)
        with tc.tile_pool(name="f1_ps", bufs=4, space="PSUM") as psF1:
            for q in range(4):
                s5 = ts(q, TQ)
                for m in range(FC):
                    h_ps = psF1.tile([P, TQ], f32, name="h_ps", tag="h")
                    for cp in range(2):
                        nc.tensor.matmul(
                            h_ps,
                            lhsT=w18_sb[:, cp, :, ts(m, P)],
                            rhs=ln18_sb[:, cp, :, s5],
                            start=(cp == 0), stop=(cp == 1),
                            perf_mode=DR, skip_group_check=True,
                        )
                    if m % 2 == 0:
                        nc.scalar.activation(
                            hid_sb[:, m, s5], h_ps, AF.Relu,
                            bias=b1pT_sb[:, m : m + 1], scale=1.0,
                        )
                    else:
                        nc.vector.tensor_scalar(
                            hid_sb[:, m, s5], h_ps, b1pT_sb[:, m : m + 1],
                            0.0, ALU.add, ALU.max)

        # FFN2 (bf16) + LN2, quarter-granularity pipeline
        with tc.tile_pool(name="f2_ps", bufs=4, space="PSUM") as psF2, \
             tc.tile_pool(name="ln2s_ps", bufs=1, space="PSUM") as psS2:
            sum2_ps = psS2.tile([1, TOK], f32, name="ln2_sum")
            sq2_ps = psS2.tile([1, TOK], f32, name="ln2_sqsum")
            for q in range(4):
                s5 = ts(q, TQ)
                for m in range(DC):
                    f_ps = psF2.tile([P, TQ], f32, name="f_ps", tag="f")
                    for c in range(FC):
                        nc.tensor.matmul(
                            f_ps,
                            lhsT=w2_sb[:, c, ts(m, P)],
                            rhs=hid_sb[:, c, s5],
                            start=(c == 0), stop=(c == FC - 1),
                            skip_group_check=True,
                        )
                    nc.vector.scalar_tensor_tensor(
                        out=spine_sb[:, m, s5], in0=f_ps,
                        scalar=b2pT_sb[:, m : m + 1], in1=ln1g_sb[:, m, s5],
                        op0=ALU.add, op1=ALU.add,
                    )
                    emit_stats(spine_sb[:, m, s5], sum2_ps, sq2_ps, s5,
                               m == 0, m == DC - 1, "s2")
                _ln_chain(nc, ln2t, sum2_ps, sq2_ps, eps_sb, [(q * TQ, TQ)])
                for c in range(DC):
                    v = workP.tile([P, TQ], bf16, name="ln2v", tag="ln2v")
                    nc.vector.tensor_mul(v, spine_sb[:, c, s5],
                                         ln2t["bcA"][:, s5])
                    t = workP.tile([P, TQ], bf16, name="ln2t", tag="ln2t")
                    nc.vector.tensor_tensor(t, v, ln2t["bcB"][:, s5],
                                            op=ALU.add)
                    eng = nc.vector if c % 2 == 0 else nc.gpsimd
                    eng.tensor_scalar(
                        out_sb[:, c, s5], t, g2T_sb[:, c : c + 1],
                        beta2T_sb[:, c : c + 1], ALU.mult, ALU.add)
                    nc.sync.dma_start(out=outT_d[:, c, s5],
                                      in_=out_sb[:, c, s5])

    if not nc.is_finalized():
        nc.finalize()
    return nc


def _prep_inputs(x, Wqkv, bqkv, Wo, bo, g1, beta1, W1, b1, W2, b2, g2, beta2):
    """Host-side sharding/layout prep -> list of 8 in_maps."""
    f = lambda a: np.ascontiguousarray(np.asarray(a, dtype=np.float32))
    bf = lambda a: np.ascontiguousarray(
        np.asarray(a, dtype=np.float32).astype(ml_dtypes.bfloat16))
    q8 = lambda a: np.ascontiguousarray(
        np.asarray(a, dtype=np.float32).astype(F8NP))

    def pack8(w):  # [512, N] -> [128, 2, 2, N] fp8, c = 2*cp + i
        w = np.asarray(w, dtype=np.float32)
        return q8(w.reshape(2, 2, P, w.shape[1]).transpose(2, 0, 1, 3))

    def chunkT(w, nchunk, cast):  # [n*128, cols] -> [128, n, cols]
        w = np.asarray(w, dtype=np.float32)
        return cast(w.reshape(nchunk, P, w.shape[1]).transpose(1, 0, 2))

    Wqkv = np.asarray(Wqkv, np.float32)
    s2 = 1.0 / np.sqrt(2.0)
    bqkv_s = np.asarray(bqkv, np.float32).copy()
    bqkv_s[: 2 * D] *= s2                      # q,k bias pre-scaled
    b1p = np.asarray(b1, np.float32) + np.asarray(beta1, np.float32) @ np.asarray(W1, np.float32)
    b2p = np.asarray(b2, np.float32) + np.asarray(beta1, np.float32)
    shared = {
        "wq8": pack8(Wqkv[:, 0:D] * s2),
        "wk8": pack8(Wqkv[:, D : 2 * D] * s2),
        "wv8": pack8(Wqkv[:, 2 * D :]),
        "wo8": pack8(Wo),
        "w18": pack8(np.asarray(W1, np.float32)
                     * np.asarray(g1, np.float32)[:, None] * 16.0),
        "w2": chunkT(np.asarray(W2, np.float32) / 16.0, FC, bf),
        "bqkvT": f(bqkv_s.reshape(12, P).T),
        "boT": f(np.asarray(bo).reshape(DC, P).T),
        "b1pT": f(b1p.reshape(FC, P).T * 16.0),
        "b2pT": f(b2p.reshape(DC, P).T),
        "bvrow": f(np.asarray(bqkv, np.float32)[2 * D :].reshape(1, D)),
        "ones_col": np.ones((P, 1), ml_dtypes.bfloat16),
        "g1T": f(np.asarray(g1).reshape(DC, P).T),
        "g2T": f(np.asarray(g2).reshape(DC, P).T),
        "beta2T": f(np.asarray(beta2).reshape(DC, P).T),
    }
    x = np.asarray(x, dtype=np.float32)
    in_maps = []
    for c in range(8):
        b, half = c // 2, c % 2
        xbT = x[b].T.reshape(2, 2, P, S).transpose(2, 0, 1, 3)   # [128,2,2,S]
        xq = x[b, half * TOK : (half + 1) * TOK]
        xqT4 = xq.T.reshape(DC, P, TOK).transpose(1, 0, 2)        # [128,4,TOK]
        x8qT = xq.T.reshape(2, 2, P, TOK).transpose(2, 0, 1, 3)
        in_maps.append(dict(
            shared, x8b=q8(xbT), x8q=q8(x8qT), xq=bf(xqT4)))
    return in_maps


def kernel(**inputs):
    from concourse.bass_utils import run_bass_kernel_spmd

    nc = _build_program()
    in_maps = _prep_inputs(**inputs)
    res = run_bass_kernel_spmd(nc, in_maps, core_ids=list(range(8)))
    out = np.empty((B, S, D), dtype=np.float32)
    for c in range(8):
        b, half = c // 2, c % 2
        oT = np.asarray(res.results[c]["outT"], dtype=np.float32)  # [P,DC,TOK]
        out[b, half * TOK : (half + 1) * TOK] = (
            oT.transpose(2, 1, 0).reshape(TOK, D)
        )
    return out
